# revision 5
# baseline (speedup 1.0000x reference)
"""Trainium2 Bass kernel for nn_DihedralGroupConv.

Math: reference computes
    filt[c,i,d,o] = sum_g perm[g,i,o] * weight[g,c,d]
    out = x.reshape(B,-1) @ filt.reshape(C*2n, D*2n)
i.e. out[b,d,o] = sum_{g,c} weight[g,c,d] * x[b,c, idx_g(o)]
where perm[g] are dihedral regular-rep permutation matrices: half-wise
cyclic shifts of x (rotations) or of the reflected array (reflections).

Design (from ntff trace analysis; data-parallel, 64 batch per core):
  - Host builds one flat per-core image [128, 7424] fp16: per-gen
    BLOCK-DIAGONAL weight tiles (W[g] on the four diagonal 32x32
    blocks) followed by the halo-padded per-half image of x with
    partition = 32*(b%4) + c.
  - One FULL-ARRAY 128x128 matmul per (gen, quad): K=(b%4,c)=128
    against the block-diagonal stationary, M=(b%4,d)=128, streaming
    400 columns; rotations read a shifted window forward, reflections
    a shifted window backward (negative-stride AP with swapped
    halves).  All 4 gens accumulate into one PSUM bank per quad.
    Measured cadence is the 170ns/400-col floor: LDWEIGHTS (~80ns,
    reloaded per matmul by the framework) hides completely under the
    previous matmul's retire window, so gen-major pair-blocks cost
    nothing extra and let each pair drain early.
  - Input streams over BOTH HWDGE queues (sync + scalar engines) in
    consumption order: the 16 SDMA engines (~22GB/s each) are shared
    between the rings and round-robin per line, so a chunk's
    completion sem fires only after everything queued ahead of it on
    either ring has moved.  Output DMAs (per quad-pair) alternate
    queues.  Single-queue measured ~182GB/s with in+out mixed; split
    ~305GB/s aggregate.
  - fp32 warm-up matmuls sized to end just past the first chunk's
    arrival keep the PE HAM activity window open (any >350ns idle gap
    resets it and costs ~2us of 1.2GHz matmuls); fp8 fails the 2e-2
    gate (3.5e-2) so fp16 it is.
  - Tail: quads 14/15 run as half-column blocks, drains on vector
    (scalar's queue would serialize them behind its DMA descriptor
    emissions), stores split across both queues, so the exposed work
    after the last matmul is one [128,200] drain + one store.
  - The NEFF epilogue clears all 256 semaphores per-engine (fixed
    ~7us, Tensor chain 53 x 115ns dominates, cadence independent of
    the PE clock) — only the body end time is controllable.
All DMAs are pure contiguous-run transfers; the host unscrambles the
[128, 6400] fp16 output image.
"""

import numpy as np

import concourse.bass as bass
import concourse.mybir as mybir
from concourse import bacc
from concourse.tile import TileContext
from concourse.bass_utils import run_bass_kernel_spmd

# Problem constants (hardcoded per harness contract).
B = 512
C = 32          # in channels
D = 32          # out channels
N = 200         # half length; 2N = 400
L = 2 * N
N_CORES = 8
BPC = B // N_CORES          # 64 batch per core
NQ = BPC // 4               # 16 quads (4 batch / quad)
HALO = 8
PH = N + 2 * HALO           # 216 padded half length
QW = 2 * PH                 # flat columns per quad (432)

G = 4                       # generators
WS0 = G * 128               # block-diag weight cols prepended (512)

_DT_IN = mybir.dt.float16
_DT_OUT = mybir.dt.float16
_NP_IN = np.float16

# --- tunables -----------------------------------------------------------
# gen-major quad blocks; a block needs all its quads' data resident.
# LDWEIGHTS is fully hidden under the previous matmul's retire window
# (measured 170ns/MM cadence = the 400-col floor), so small blocks are
# free; per-pair blocks give the earliest data consumption and smooth
# drain pacing.  Quads 14/15 run solo for a short tail.
BLOCKS = [(0,), (1,), (2, 3), (4, 5), (6, 7), (8, 9), (10, 11), (12, 13)]
# trailing keep-warm matmuls: measured useless (the epilogue's semaphore
# clear cadence is 115ns/op independent of the PE clock) — disabled.
WARM_KEEPERS = 0
# initial fp32 warm-up matmul calls (each lowers to 2 passes, ~1.5us cold)
WARMUP_CALLS = 1
# second warm-up call length: sized so the warm-up burst ends just PAST
# the first chunk's completion (~9.3us with the partition-split early
# chunks) — an idle PE gap >350ns before the real stream resets the HAM
# activity window and costs ~2us of half-clock matmuls (observed in v4);
# overshooting delays the whole stream (FIFO) — measured 0.7us lost
# with 400 here.
WARMUP_COLS2 = 288

_cache = {}


def _derive_gens(perm):
    """Classify each generator as (is_refl, shift s); see v1."""
    n = N
    o = np.arange(n)
    gens = []
    for g in range(perm.shape[0]):
        idx = np.argmax(perm[g], axis=0).astype(np.int64)  # y[o] = x[idx[o]]
        r = int((-idx[0]) % n)
        rot = np.concatenate([(o - r) % n, n + (o - r) % n])
        if np.array_equal(idx, rot):
            s = -r if r <= n // 2 else n - r
            gens.append((False, s))
            continue
        r = int(idx[0] - n) % n
        r = (-r) % n
        refl = np.concatenate([n + (-o - r) % n, (-o - r) % n])
        if np.array_equal(idx, refl):
            s = r if r <= n // 2 else r - n
            gens.append((True, s))
            continue
        raise NotImplementedError(f"perm[{g}] is not a dihedral rep matrix")
    for is_refl, s in gens:
        if is_refl:
            ok = -(HALO - 1) <= s <= HALO
        else:
            ok = -HALO <= s <= HALO
        if not ok:
            raise NotImplementedError(f"shift {s} exceeds halo {HALO}")
    return gens


def _layout():
    """Flat-image column layout: [ws_g0 | q0 | ws_g1..g3 | q1 | q2.. ]."""
    cols = {}
    c = 0
    cols['ws0'] = c; c += 128              # gen 0 weights
    cols['q0'] = c; c += QW
    cols['ws_rest'] = c; c += (G - 1) * 128
    for q in range(1, NQ):
        cols[f'q{q}'] = c; c += QW
    return cols, c


_COLS, TOT = _layout()


def _qcol(q):
    return _COLS[f'q{q}']


def _wcol(i):
    return _COLS['ws0'] if i == 0 else _COLS['ws_rest'] + (i - 1) * 128


def _build_program(gens):
    rot = [(j, s) for j, (is_r, s) in enumerate(gens) if not is_r]
    refl = [(j, s) for j, (is_r, s) in enumerate(gens) if is_r]
    ng = len(rot) + len(refl)
    assert ng == G

    nc = bacc.Bacc("TRN2", target_bir_lowering=False, debug=False,
                   num_devices=N_CORES, enable_partition_id=False)
    # This kernel never issues gpsimd (SWDGE) DMAs; dropping the unused
    # qPoolDynamic declaration lets the NEFF allocate fewer DMA-queue
    # semaphores, shortening the fixed per-engine semaphore-clear
    # epilogue (the Tensor chain, 53 x 115ns, is the teardown long pole).
    nc.m.queues = [q for q in nc.m.queues
                   if not q.name.startswith("qPoolDynamic")]
    ax_d = nc.dram_tensor("ax", [128, TOT], _DT_IN, kind="ExternalInput")
    outr_d = nc.dram_tensor("outr", [128, NQ * L], _DT_OUT,
                            kind="ExternalOutput")

    # input chunks per queue as (col_start, col_end, part_start,
    # part_end), in consumption order.  The 16 SDMA engines are shared
    # by both rings and round-robin per line, so a chunk's completion
    # sem fires only after everything queued ahead of it on either ring
    # has moved.  The three early pieces the cold-clock stream consumes
    # back-to-back (ws_g0+q0, ws_g1..g3, q1) are each split BY
    # PARTITIONS across the rings: partitions 0-63 ride engines 64-71
    # and 64-127 ride 72-79, so the halves move on disjoint engines in
    # parallel and complete ~2us earlier than queue-ordered whole
    # chunks (measured: whole q1 sem fired at 12.2us, stalling block 1
    # ~1us and risking a HAM window reset).
    # Each chunk-completion sem needs ALL 16 SDMA engines to finish its
    # share, and the engines are shared with the partner NeuronCore —
    # one straggler engine delays the whole sem by up to ~2us (observed:
    # a ws-chunk sem fired at 12.6us, stalling the stream's 2nd matmul).
    # So MINIMIZE the number of sem gates on the critical path: ws_g0 +
    # q0 + ws_g1..g3 are contiguous in the layout and ride as ONE chunk
    # per ring (partition-split on disjoint engine halves).
    c1_end = _COLS['ws_rest'] + (G - 1) * 128        # ws0|q0|ws_rest
    sync_chunks = [
        (0, c1_end, 0, 64),                          # ws+q0, parts 0-63
        (_qcol(1), _qcol(1) + QW, 0, 64),            # q1, parts 0-63
        (_qcol(2), _qcol(2) + QW, 0, 128),           # q2
        (_qcol(4), _qcol(4) + 2 * QW, 0, 128),       # q4,5
        (_qcol(8), _qcol(8) + 2 * QW, 0, 128),       # q8,9
        (_qcol(12), _qcol(12) + 2 * QW, 0, 128),     # q12,13
        (_qcol(15), _qcol(15) + QW, 0, 128),         # q15
    ]
    scalar_chunks = [
        (0, c1_end, 64, 128),                        # ws+q0, parts 64-127
        (_qcol(1), _qcol(1) + QW, 64, 128),          # q1, parts 64-127
        (_qcol(3), _qcol(3) + QW, 0, 128),           # q3
        (_qcol(6), _qcol(6) + 2 * QW, 0, 128),       # q6,7
        (_qcol(10), _qcol(10) + 2 * QW, 0, 128),     # q10,11
        (_qcol(14), _qcol(14) + QW, 0, 128),         # q14
    ]
    # full coverage: every column covered across all 128 partitions
    col_parts = {}
    for a, b, p0, p1 in sync_chunks + scalar_chunks:
        for c in range(a, b):
            col_parts[c] = col_parts.get(c, 0) + (p1 - p0)
    assert all(col_parts.get(c, 0) == 128 for c in range(TOT)), \
        "chunks must tile the image"

    with TileContext(nc) as tc:
        with (
            tc.tile_pool(name="arrp", bufs=1) as arrp,
            tc.tile_pool(name="stg", bufs=1) as stgp,
            tc.tile_pool(name="psum", bufs=1, space="PSUM") as psump,
        ):
            ax_sb = arrp.tile([128, TOT], _DT_IN, name="ax_sb")
            # interleave issue order: sync first (carries the critical
            # ws_g0+q0), then alternate so both rings fill early.
            order = []
            for k in range(max(len(sync_chunks), len(scalar_chunks))):
                if k < len(sync_chunks):
                    order.append((nc.sync, sync_chunks[k]))
                if k < len(scalar_chunks):
                    order.append((nc.scalar, scalar_chunks[k]))
            for eng, (a, b, p0, p1) in order:
                eng.dma_start(out=ax_sb[p0:p1, a:b], in_=ax_d[p0:p1, a:b])

            pstiles = [psump.tile([128, L], mybir.dt.float32,
                                  name=f"ps{i}") for i in range(8)]
            stgs = [stgp.tile([128, 2, L], _DT_OUT, name=f"stg{i}")
                    for i in range(4)]
            # separate half-tiles for the split tail quads
            tl14 = stgp.tile([128, L], _DT_OUT, name="tl14")
            tl15 = stgp.tile([128, L], _DT_OUT, name="tl15")

            # HAM warm-up (see v1): fp32 zero-broadcast matmuls from the
            # framework const pool; no Tile dependency needed.
            zc = nc.const_aps.aps[(mybir.dt.float32, 0.0)]
            wu_lhs = bass.AP(zc.tensor, 0, [[zc.ap[0][0], 128], [0, 128]])
            wu_rhs = bass.AP(zc.tensor, 0, [[zc.ap[0][0], 128], [0, 400]])
            for _ in range(WARMUP_CALLS):
                nc.tensor.matmul(pstiles[7][:, 0:400], wu_lhs, wu_rhs,
                                 start=True, stop=True,
                                 skip_group_check=True)
            if WARMUP_COLS2:
                wu_rhs2 = bass.AP(zc.tensor, 0,
                                  [[zc.ap[0][0], 128], [0, WARMUP_COLS2]])
                nc.tensor.matmul(pstiles[7][:, 0:WARMUP_COLS2], wu_lhs,
                                 wu_rhs2, start=True, stop=True,
                                 skip_group_check=True)

            # (is_refl, weight col index, window param) in ws order
            mm_descs = []
            for k, (_, s) in enumerate(rot):
                mm_descs.append((False, k, s + HALO))
            for k, (_, s) in enumerate(refl):
                mm_descs.append((True, len(rot) + k, s))

            axt = ax_sb[:, :]
            pstride = axt.ap[0][0]

            def rhs_ap(q, is_r, w):
                base = _qcol(q)
                if not is_r:
                    return bass.AP(axt.tensor, base + w,
                                   [[pstride, 128], [PH, 2], [1, N]])
                off = base + PH + (PH - HALO - w)
                return bass.AP(axt.tensor, off,
                               [[pstride, 128], [-PH, 2], [-1, N]])

            def rhs_ap_half(q, is_r, w, h):
                # single output half h (N columns), for the tail split
                base = _qcol(q)
                if not is_r:
                    return bass.AP(axt.tensor, base + h * PH + w,
                                   [[pstride, 128], [1, N]])
                off = base + (1 - h) * PH + (PH - HALO - w)
                return bass.AP(axt.tensor, off,
                               [[pstride, 128], [-1, N]])

            def mm(q, i):
                is_r, wi, w = mm_descs[i]
                wc = _wcol(wi)
                nc.tensor.matmul(
                    pstiles[q % 8][:, :],
                    ax_sb[:, wc:wc + 128],
                    rhs_ap(q, is_r, w),
                    start=(i == 0), stop=(i == ng - 1),
                    skip_group_check=True,
                )

            # drains + output stores, pairwise; queue alternates per pair
            pair_done = set()

            def emit_pair(qA):
                qB = qA + 1
                k = qA // 2
                stg = stgs[k % 4]
                nc.vector.tensor_copy(out=stg[:, 0],
                                      in_=pstiles[qA % 8][:, :])
                nc.scalar.copy(out=stg[:, 1], in_=pstiles[qB % 8][:, :])
                eng = nc.sync if (k % 2 == 0) else nc.scalar
                eng.dma_start(out=outr_d[:, qA * L:(qB + 1) * L],
                              in_=stg[:, :, :])

            for blk in BLOCKS:
                for i in range(ng):
                    for q in blk:
                        mm(q, i)
                # drain pairs whose both quads are complete (tail pair
                # (14,15) is handled separately below)
                hi = blk[-1]
                for qA in range(0, NQ - 2, 2):
                    if qA + 1 <= hi and qA not in pair_done:
                        pair_done.add(qA)
                        emit_pair(qA)

            # tail: quads 14/15 run in half-column blocks so each half's
            # drain + store overlaps the next half's matmuls; the final
            # exposed work after the last matmul is one [128,200] drain
            # plus one store.
            def mm_half(q, i, h):
                is_r, wi, w = mm_descs[i]
                wc = _wcol(wi)
                nc.tensor.matmul(
                    pstiles[q % 8][:, h * N:(h + 1) * N],
                    ax_sb[:, wc:wc + 128],
                    rhs_ap_half(q, is_r, w, h),
                    start=(i == 0), stop=(i == ng - 1),
                    skip_group_check=True,
                )

            # All tail drains on vector (scalar's queue would serialize
            # them behind its DMA descriptor emissions); stores split
            # across both queues.
            tl = {14: tl14, 15: tl15}
            for q in (14, 15):
                for h in (0, 1):
                    for i in range(ng):
                        mm_half(q, i, h)
                    nc.vector.tensor_copy(
                        out=tl[q][:, h * N:(h + 1) * N],
                        in_=pstiles[q % 8][:, h * N:(h + 1) * N])
                    if q == 15 and h == 0:
                        # store h0 while h1's matmuls stream
                        nc.scalar.dma_start(
                            out=outr_d[:, 15 * L:15 * L + N],
                            in_=tl15[:, 0:N])
                if q == 14:
                    nc.sync.dma_start(out=outr_d[:, 14 * L:15 * L],
                                      in_=tl14[:, :])
            # final store split by partitions across both queues: two
            # half-size descriptor emissions run concurrently and the
            # halves move on disjoint SDMA engines
            nc.sync.dma_start(out=outr_d[0:64, 15 * L + N:16 * L],
                              in_=tl15[0:64, N:L])
            nc.scalar.dma_start(out=outr_d[64:128, 15 * L + N:16 * L],
                                in_=tl15[64:128, N:L])

            # warm-keepers: bf16 1.0-broadcast matmuls, dependency-free,
            # queued after the real stream to keep the PE active window
            # open through the drain tail and the epilogue's clear storm.
            if WARM_KEEPERS:
                oc = nc.const_aps.aps.get((mybir.dt.bfloat16, 1.0))
                if oc is not None:
                    wk_lhs = bass.AP(oc.tensor, 0,
                                     [[oc.ap[0][0], 128], [0, 128]])
                    wk_rhs = bass.AP(oc.tensor, 0,
                                     [[oc.ap[0][0], 128], [0, 400]])
                else:  # fall back to fp32 zeros (2-pass, ~333ns warm)
                    wk_lhs = bass.AP(zc.tensor, 0,
                                     [[zc.ap[0][0], 128], [0, 128]])
                    wk_rhs = bass.AP(zc.tensor, 0,
                                     [[zc.ap[0][0], 128], [0, 400]])
                for _ in range(WARM_KEEPERS):
                    nc.tensor.matmul(pstiles[0][:, :], wk_lhs, wk_rhs,
                                     start=True, stop=True,
                                     skip_group_check=True)
    nc.compile()
    return nc


def _host_images(x, weight, gens):
    """Build per-core flat images with block-diagonal weight blocks."""
    n = N
    rot = [(j, s) for j, (is_r, s) in enumerate(gens) if not is_r]
    refl = [(j, s) for j, (is_r, s) in enumerate(gens) if is_r]

    pad_idx = (np.arange(PH) - HALO) % n
    xh = x.reshape(B, C, 2, n)[:, :, :, pad_idx]          # [B, C, 2, PH]

    # block-diag weights per gen: ws_g[32u+c, 32u+d] = W_g[c, d]
    wblocks = []
    for (j, _) in rot + refl:
        blk = np.zeros((128, 128), dtype=_NP_IN)
        for u in range(4):
            blk[32 * u:32 * (u + 1), 32 * u:32 * (u + 1)] = weight[j]
        wblocks.append(blk)

    def img(a, core):
        sl = a[core * BPC:(core + 1) * BPC]               # [64, C, 2, PH]
        out = np.empty((128, TOT), dtype=_NP_IN)
        out[:, _COLS['ws0']:_COLS['ws0'] + 128] = wblocks[0]
        for k in range(1, G):
            c0 = _COLS['ws_rest'] + (k - 1) * 128
            out[:, c0:c0 + 128] = wblocks[k]
        quads = np.empty((128, NQ, QW), dtype=_NP_IN)
        for u in range(4):
            quads[32 * u:32 * (u + 1)] = (
                sl[u::4].transpose(1, 0, 2, 3).reshape(32, NQ, QW))
        for q in range(NQ):
            out[:, _qcol(q):_qcol(q) + QW] = quads[:, q]
        return np.ascontiguousarray(out)

    return [img(xh, c) for c in range(N_CORES)]


def _unscramble(outr):
    r = outr.astype(np.float32).reshape(4, D, NQ, L)    # [b%4, d, q, o]
    r = r.transpose(2, 0, 1, 3)                         # [q, b%4, d, o]
    return np.ascontiguousarray(r.reshape(BPC, D, L))


def kernel(x, weight, perm, _trace=False):
    x = np.asarray(x, dtype=np.float32)
    weight = np.asarray(weight, dtype=np.float32)
    perm = np.asarray(perm, dtype=np.float32)

    gens = _derive_gens(perm)
    key = tuple(gens)
    if key not in _cache:
        _cache[key] = _build_program(gens)
    nc = _cache[key]

    axs = _host_images(x, weight, gens)
    in_maps = [{"ax": axs[c]} for c in range(N_CORES)]
    res = run_bass_kernel_spmd(nc, in_maps, core_ids=list(range(N_CORES)),
                               trace=_trace)
    out = np.concatenate([_unscramble(res.results[c]["outr"])
                          for c in range(N_CORES)], axis=0)
    if _trace:
        kernel.last_exec_time_ns = res.exec_time_ns
        kernel.last_results = res
    return out
